# revision 54
# baseline (speedup 1.0000x reference)
"""Trainium2 Bass kernel for nn_DebedderNeuronGroup_index.

Math (per layer l, kn=KN[l], ksci=KS[l]*CI[l], idim=ksci+1):
    out[b, k, o] = sum_d x[b, off_l + k, d] * W_l[o, d] + b_l[o]
    y[b, S_l + k*ksci + o] = out[b, k, o]        for o <  ksci
    y[b, S_l + kn*ksci + k] = out[b, k, ksci]    (bias-column tail block)

Strategy: pure data parallelism over batch (16 per core, 8 cores), bf16
matmuls (tokens stationary on PSUM partitions, o on the free dim), f16
stores.  Trace-driven scheduling (389us baseline -> ~330us):
  - x is staged in HBM chunk-major ([128, 4*tl] per 1024-token chunk, c-major
    free dim) so every chunk load is 128 descriptors of 8KB instead of 512 of
    2KB; 2KB descriptors capped the x queue at ~78GB/s and starved the PE.
  - The bias column is folded into the main o-tiling (o-tiles cover idim =
    ksci+1, equal-split <=512), removing a separate column pass (~19us of PE
    streaming).  L3's per-subtile bias-column values are extracted with
    scalar-engine copies, stored once, and scattered into y on the host.
  - L0/L4 (and L1 whole, L2 in 8-subtile groups) accumulate in SBUF and go
    to DRAM scratches as a few huge-descriptor stores; the host reorders
    them into y's layout.  Their direct y-layout stores were thousands of
    1-2KB descriptors that saturate a DMA queue's ~25ns/descriptor rate.
  - Processing order L0,L4,L1,L2,L3; all loads ride the sync+scalar queues
    in exact need order (a queue is FIFO, so ordering = priority; two queues
    pull more aggregate HBM bandwidth than one), with WT3 and late x chunks
    issued from prefetch points inside the loop.  L3 stores split halves
    across the scalar/gpsimd queues; the last subtile stores in quarters.
  - 64 warmup + 16 filler matmuls on a zeroed tile keep the PE busy across
    load-latency windows so the HAM clock gate stays at full speed.
"""

import numpy as np
import ml_dtypes

import concourse.mybir as mybir
from concourse import bacc
from concourse.tile import TileContext
from concourse.bass_utils import run_bass_kernel_spmd

# ---------------------------------------------------------------- constants
N_CORES = 8
B = 128
BPC = B // N_CORES            # batches per core = 16
D = 512
KN = [64, 128, 256, 256, 10]
KSCI = [27, 576, 1152, 4096, 256]
IDIM = [k + 1 for k in KSCI]
START = [0, 1792, 75648, 370816, 1419648]
I_TOTAL = 1422218
TOKL = [BPC * k for k in KN]  # tokens per core per layer
NTOK = sum(TOKL)              # 11424
BBOFF = np.cumsum([0] + IDIM).tolist()  # bias table offset per layer
BBTOT = BBOFF[-1]             # 6112
TLOAD = 1024                  # tokens per x DMA chunk
BF16 = mybir.dt.bfloat16
F16 = mybir.dt.float16
F32 = mybir.dt.float32

# o-tile widths per layer: equal split of idim into ceil(idim/512) tiles
def _otw(idim):
    nt = -(-idim // 512)
    base, rem = divmod(idim, nt)
    return [base + 1] * rem + [base] * (nt - rem)
OTW = [_otw(i) for i in IDIM]

# x chunk schedule: (layer, t0, tl) in issue (= need) order.  L0's chunk is
# split in two so the first matmul waits on 0.5MB, not 1MB.
CH = [
    (0, 0, 512), (0, 512, 512), (4, 0, 160), (1, 0, 1024), (1, 1024, 1024),
    (2, 0, 1024), (2, 1024, 1024), (3, 0, 1024), (2, 2048, 1024),
    (3, 1024, 1024), (2, 3072, 1024), (3, 2048, 1024), (3, 3072, 1024),
]
CHIDX = {2: [5, 6, 8, 10], 3: [7, 9, 11, 12]}
XFOFF = np.cumsum([0] + [4 * tl for (_, _, tl) in CH]).tolist()
XFTOT = XFOFF[-1]             # 45696
# phase B subtile order: all of L2, then all of L3 (stores are split across
# two queues, so neither phase bursts past queue bandwidth; serial order
# maximizes the deadline slack for WT3 and L3's x chunks)
SCHED = [(2, i) for i in range(32)] + [(3, i) for i in range(32)]
# prefetch points: at (l, s) issue loads for these CH indices (None = WT3)
PFCH = {
    (2, 4): [6],              # L2c1
    (2, 12): [None, 8, 7],    # WT3, L2c2, L3c0
    (2, 20): [10],            # L2c3
    (3, 4): [9],              # L3c1
    (3, 12): [11],            # L3c2
    (3, 20): [12],            # L3c3
}

# bias-column scratch: L3 subtiles only (L1/L2 columns ride their scratches)
NCOLS = 32

_cache = {}
last_results = None


def _build_bass():
    nc = bacc.Bacc(
        "TRN2", target_bir_lowering=False, debug=False, num_devices=N_CORES
    )
    XF = nc.declare_dram_parameter("XF", [128, XFTOT], BF16, isOutput=False)
    WT = [
        nc.declare_dram_parameter(f"WT{l}", [128, 4 * IDIM[l]], BF16, isOutput=False)
        for l in range(5)
    ]
    BBp = nc.declare_dram_parameter("BB", [128, BBTOT], BF16, isOutput=False)
    y = nc.declare_dram_parameter("y", [BPC, I_TOTAL], F16, isOutput=True)
    Y0S = nc.declare_dram_parameter("Y0S", [128, 8 * 28], F16, isOutput=True)
    Y4S = nc.declare_dram_parameter("Y4S", [128, 2 * 257], F16, isOutput=True)
    Y1S = nc.declare_dram_parameter("Y1S", [128, 16 * 577], F16, isOutput=True)
    Y2S = nc.declare_dram_parameter("Y2S", [128, 32 * 1153], F16, isOutput=True)
    YCOL = nc.declare_dram_parameter("YCOL", [128, NCOLS], F16, isOutput=True)

    with TileContext(nc) as tc:
        with (
            tc.tile_pool(name="wt", bufs=1) as wt_pool,
            tc.tile_pool(name="bias", bufs=1) as bias_pool,
            tc.tile_pool(name="x", bufs=5) as x_pool,
            tc.tile_pool(name="outL", bufs=5) as outL_pool,
            tc.tile_pool(name="outS", bufs=2) as outS_pool,
            tc.tile_pool(name="fix", bufs=1) as fix_pool,
            tc.tile_pool(name="ps", bufs=7, space="PSUM") as ps_pool,
            tc.tile_pool(name="pw", bufs=1, space="PSUM") as pw_pool,
        ):
            # ---- PE warmup: keep the HAM activity monitor fed while the
            # first x chunk loads, so real matmuls start at full clock.
            warm = fix_pool.tile([128, 128], BF16, tag="warm")
            nc.vector.memset(warm[:, :], 0.0)
            wps = pw_pool.tile([128, 128], F32, tag="wps")
            for _ in range(64):
                nc.tensor.matmul(
                    out=wps[:, :], lhsT=warm[:, :], rhs=warm[:, :],
                    start=True, stop=True,
                )

            # ---- every load (x chunks AND tables) goes on the sync-ring
            # queue, interleaved in exact need order.  Cross-queue HBM
            # arbitration is uncontrollable; a single FIFO queue is an
            # explicit priority list.  Stores get the other two queues.
            bb = bias_pool.tile([128, BBTOT], BF16, tag="bb")
            wt = {}

            def load_tables(l, eng=None):
                eng = eng or nc.sync
                t = wt_pool.tile([128, 4 * IDIM[l]], BF16, tag=f"wt{l}")
                eng.dma_start(out=t[:, :], in_=WT[l][:, :])
                eng.dma_start(
                    out=bb[:, BBOFF[l] : BBOFF[l] + IDIM[l]],
                    in_=BBp[:, BBOFF[l] : BBOFF[l] + IDIM[l]],
                )
                wt[l] = t[:, :].rearrange("p (c o) -> p c o", c=4)

            colbuf = fix_pool.tile([128, NCOLS], F16, tag="colbuf")
            ob0 = fix_pool.tile([128, 8 * 28], F16, tag="ob0")
            ob4 = fix_pool.tile([128, 2 * 257], F16, tag="ob4")
            ob1 = fix_pool.tile([128, 16 * 577], F16, tag="ob1")
            nc.vector.memset(ob4[:, :], 0.0)  # rows >= sl never written

            ymain3 = y[:, START[3] : START[3] + KN[3] * KSCI[3]].rearrange(
                "b (k o) -> b k o", o=KSCI[3]
            )

            def load_chunk(ci, eng=None):
                eng = eng or nc.sync
                _, _, tl = CH[ci]
                xt = x_pool.tile([128, 4 * TLOAD], BF16, tag="xt")
                eng.dma_start(
                    out=xt[:, : 4 * tl], in_=XF[:, XFOFF[ci] : XFOFF[ci] + 4 * tl]
                )
                return xt[:, : 4 * tl].rearrange("p (c t) -> p c t", c=4)

            def subtile_mm(l, xv, s0, sl, drain):
                o0 = 0
                for oi, no in enumerate(OTW[l]):
                    ps = ps_pool.tile([128, 512], F32, tag="ps")
                    for dc in range(4):
                        nc.tensor.matmul(
                            out=ps[:sl, :no],
                            lhsT=xv[:, dc, s0 : s0 + sl],
                            rhs=wt[l][:, dc, o0 : o0 + no],
                            start=(dc == 0),
                            stop=(dc == 3),
                        )
                    drain(oi, o0, no, ps)
                    o0 += no

            def badd(out_ap, ps, sl, no, l, o0):
                nc.vector.tensor_add(
                    out=out_ap,
                    in0=ps[:sl, :no],
                    in1=bb[:sl, BBOFF[l] + o0 : BBOFF[l] + o0 + no],
                )

            # ---- issue ALL loads upfront on the sync ring in need order.
            # The x pool (bufs=6) lets the first 6 chunk triggers fire
            # immediately; later chunk triggers wait for buffer releases,
            # which track compute progress.  Table triggers fire eagerly and
            # their transfers serialize in this exact order on the queue.
            # Early loads are sharded across the sync AND scalar queues (the
            # scalar queue has no stores until Y1S at ~45us): two queues pull
            # measurably more aggregate HBM bandwidth than one, and phase A
            # is load-latency-bound.
            xtile = {}
            def pf(ci, eng=None):
                xtile[ci] = load_chunk(ci, eng)
            pf(0)                               # L0 x 1st half (sync)
            pf(1)                               # L0 x 2nd half (sync)
            load_tables(0, nc.scalar)
            load_tables(4, nc.scalar)
            pf(2, nc.scalar)                    # L4 x
            load_tables(1)                      # WT1           (sync)
            pf(3, nc.scalar)                    # L1 x 1st half
            pf(4)                               # L1 x 2nd half (sync)
            load_tables(2, nc.scalar)
            pf(5)                               # L2 chunk 0    (sync)
            # WT3 and the remaining chunks are issued from inside the phase-B
            # loop (PFCH) so the queues stay in need order.

            # ---- phase A: L0 -> scratch
            for s in range(8):
                def dr0(oi, o0, no, ps, s=s):
                    badd(ob0[:128, s * 28 + o0 : s * 28 + o0 + no], ps, 128, no, 0, o0)
                subtile_mm(0, xtile[s // 4], (s % 4) * 128, 128, dr0)
            nc.gpsimd.dma_start(out=Y0S[:, :], in_=ob0[:, :])

            # ---- L4 -> scratch (2 subtiles: 120 + 40 tokens)
            for si, (s0, sl) in enumerate(((0, 120), (120, 40))):
                def dr4(oi, o0, no, ps, si=si, sl=sl):
                    badd(
                        ob4[:sl, si * 257 + o0 : si * 257 + o0 + no],
                        ps, sl, no, 4, o0,
                    )
                subtile_mm(4, xtile[2], s0, sl, dr4)
            nc.gpsimd.dma_start(out=Y4S[:, :], in_=ob4[:, :])

            # ---- filler: L1's x arrival is HBM-bound ~1-2us after L4 ends;
            # keep the PE busy so the HAM clock gate stays at full speed.
            for _ in range(16):
                nc.tensor.matmul(
                    out=wps[:, :], lhsT=warm[:, :], rhs=warm[:, :],
                    start=True, stop=True,
                )

            # ---- L1 -> scratch (accumulated whole: 1 store of 18.5KB lines
            # instead of 2048 1.2KB store descriptors that clog the queues)
            for ch in range(2):
                xv = xtile[3 + ch]
                for si in range(8):
                    s = ch * 8 + si
                    def dr1(oi, o0, no, ps, s=s):
                        badd(
                            ob1[:128, s * 577 + o0 : s * 577 + o0 + no],
                            ps, 128, no, 1, o0,
                        )
                    subtile_mm(1, xv, si * 128, 128, dr1)
            nc.scalar.dma_start(out=Y1S[:, :], in_=ob1[:, :])

            # ---- phase B: all of L2 (scratch, 8-subtile groups), then all
            # of L3 (direct y stores in halves across both store queues).
            xcur = {}
            for l, s in SCHED:
                if s % 8 == 0:
                    xcur[l] = xtile[CHIDX[l][s // 8]]
                for ci in PFCH.get((l, s), ()):
                    if ci is None:
                        load_tables(3, nc.scalar)
                    else:
                        pf(ci)
                if l == 2:
                    j = s % 8
                    if j == 0:
                        ob2 = outS_pool.tile([128, 8 * IDIM[2]], F16, tag="ob2")
                    def dr2(oi, o0, no, ps, ob2=ob2, j=j):
                        badd(
                            ob2[:128, j * 1153 + o0 : j * 1153 + o0 + no],
                            ps, 128, no, 2, o0,
                        )
                    subtile_mm(2, xcur[2], j * 128, 128, dr2)
                    if j == 7:
                        g = s // 8
                        eng = nc.scalar if g % 2 == 0 else nc.gpsimd
                        eng.dma_start(
                            out=Y2S[:, g * 9224 : (g + 1) * 9224], in_=ob2[:, :]
                        )
                else:
                    b0, k0 = divmod(s * 128, KN[3])
                    enA = nc.scalar if s % 2 == 0 else nc.gpsimd
                    enB = nc.gpsimd if s % 2 == 0 else nc.scalar
                    ob3 = outL_pool.tile([128, IDIM[3]], F16, tag="ob3")
                    if s < 31:
                        def dr3(oi, o0, no, ps, ob3=ob3, b0=b0, k0=k0, enA=enA):
                            badd(ob3[:128, o0 : o0 + no], ps, 128, no, 3, o0)
                            if oi == 4:  # cols [0, 2277) drained
                                enA.dma_start(
                                    out=ymain3[b0, k0 : k0 + 128, 0:2048],
                                    in_=ob3[:, 0:2048],
                                )
                        subtile_mm(3, xcur[3], (s % 8) * 128, 128, dr3)
                        enB.dma_start(
                            out=ymain3[b0, k0 : k0 + 128, 2048:4096],
                            in_=ob3[:, 2048:4096],
                        )
                    else:
                        # last subtile: fine-grained stores to shorten the
                        # post-matmul drain tail
                        QS = {2: (0, 1024, enA), 4: (1024, 2048, enB),
                              6: (2048, 3072, enA), 7: (3072, 3584, enB)}
                        def dr3(oi, o0, no, ps, ob3=ob3, b0=b0, k0=k0):
                            badd(ob3[:128, o0 : o0 + no], ps, 128, no, 3, o0)
                            if oi in QS:
                                a, b, en = QS[oi]
                                en.dma_start(
                                    out=ymain3[b0, k0 : k0 + 128, a:b],
                                    in_=ob3[:, a:b],
                                )
                        subtile_mm(3, xcur[3], (s % 8) * 128, 128, dr3)
                        enB.dma_start(
                            out=ymain3[b0, k0 : k0 + 128, 3584:4096],
                            in_=ob3[:, 3584:4096],
                        )
                    nc.scalar.copy(
                        out=colbuf[:, s : s + 1],
                        in_=ob3[:, KSCI[3] : IDIM[3]],
                    )
                    if l == 3 and s == 30:
                        # all but the last column are final; store them now
                        nc.gpsimd.dma_start(
                            out=YCOL[:, 0:31], in_=colbuf[:, 0:31]
                        )
            nc.gpsimd.dma_start(out=YCOL[:, 31:32], in_=colbuf[:, 31:32])
    nc.compile()
    return nc


def _prep_inputs(inputs):
    x = np.asarray(inputs["x"], dtype=np.float32)
    xb = x.astype(ml_dtypes.bfloat16)
    shared = {}
    for l in range(5):
        W = np.asarray(inputs[f"W{l}"], dtype=np.float32).astype(ml_dtypes.bfloat16)
        # [128, 4*idim] with free dim (c, o); row p, chunk c holds W.T[c*128+p]
        shared[f"WT{l}"] = np.ascontiguousarray(
            W.T.reshape(4, 128, IDIM[l]).transpose(1, 0, 2).reshape(128, 4 * IDIM[l])
        )
    bbvec = np.concatenate(
        [np.asarray(inputs[f"b{l}"], dtype=np.float32)[: IDIM[l]] for l in range(5)]
    )
    shared["BB"] = np.ascontiguousarray(
        np.broadcast_to(bbvec.astype(ml_dtypes.bfloat16), (128, BBTOT))
    )
    off = np.cumsum([0] + KN).tolist()
    in_maps = []
    for c in range(N_CORES):
        xc = xb[c * BPC : (c + 1) * BPC]  # [16, 714, 512] bf16
        xTl = [
            np.transpose(xc[:, off[l] : off[l] + KN[l]], (2, 0, 1)).reshape(D, -1)
            for l in range(5)
        ]
        parts = []
        for l, t0, tl in CH:
            blk = xTl[l][:, t0 : t0 + tl]  # [512, tl]
            parts.append(
                blk.reshape(4, 128, tl).transpose(1, 0, 2).reshape(128, 4 * tl)
            )
        in_maps.append({"XF": np.ascontiguousarray(np.concatenate(parts, axis=1)),
                        **shared})
    return in_maps


def _assemble(res):
    y = np.empty((B, I_TOTAL), np.float32)
    for c in range(N_CORES):
        r = res.results[c]
        yc = r["y"].astype(np.float32)  # [16, I_TOTAL]
        # L0 scratch: [128, 8*28] -> tokens t=s*128+p, (b,k)=divmod(t,64)
        v0 = (
            r["Y0S"].astype(np.float32)
            .reshape(128, 8, 28).transpose(1, 0, 2).reshape(16, 64, 28)
        )
        yc[:, 0:1728] = v0[:, :, :27].reshape(16, 1728)
        yc[:, 1728:1792] = v0[:, :, 27]
        # L4 scratch: subtile 0 = tokens 0..120, subtile 1 = tokens 120..160
        v4r = r["Y4S"].astype(np.float32)  # [128, 514]
        v4 = np.concatenate(
            [v4r[:120, 0:257], v4r[:40, 257:514]], axis=0
        ).reshape(16, 10, 257)
        s4 = START[4]
        yc[:, s4 : s4 + 2560] = v4[:, :, :256].reshape(16, 2560)
        yc[:, s4 + 2560 : s4 + 2570] = v4[:, :, 256]
        # L1 scratch: [128, 16*577] -> subtile s = batch s, token p = k
        v1 = (
            r["Y1S"].astype(np.float32)
            .reshape(128, 16, 577).transpose(1, 0, 2)
        )  # [16, 128, 577]
        yc[:, 1792:75520] = v1[:, :, :576].reshape(16, 73728)
        yc[:, 75520:75648] = v1[:, :, 576]
        # L2 scratch: [128, 4*(8*1153)] -> token t = (8g+j)*128+p
        v2 = (
            r["Y2S"].astype(np.float32)
            .reshape(128, 4, 8, 1153).transpose(1, 2, 0, 3).reshape(16, 256, 1153)
        )
        yc[:, 75648:370560] = v2[:, :, :1152].reshape(16, 294912)
        yc[:, 370560:370816] = v2[:, :, 1152]
        # L3 bias columns
        ycol = r["YCOL"].astype(np.float32)  # [128, 32]
        yc[:, 1419392:1419648] = ycol.T.reshape(16, 256)
        y[c * BPC : (c + 1) * BPC] = yc
    return y


def kernel(**inputs):
    global last_results
    if "nc" not in _cache:
        _cache["nc"] = _build_bass()
    nc = _cache["nc"]
    in_maps = _prep_inputs(inputs)
    res = run_bass_kernel_spmd(nc, in_maps, list(range(N_CORES)))
    last_results = res
    return _assemble(res)


# revision 55
# speedup vs baseline: 1.0267x; 1.0267x over previous
"""Trainium2 Bass kernel for nn_DebedderNeuronGroup_index.

Math (per layer l, kn=KN[l], ksci=KS[l]*CI[l], idim=ksci+1):
    out[b, k, o] = sum_d x[b, off_l + k, d] * W_l[o, d] + b_l[o]
    y[b, S_l + k*ksci + o] = out[b, k, o]        for o <  ksci
    y[b, S_l + kn*ksci + k] = out[b, k, ksci]    (bias-column tail block)

Strategy: pure data parallelism over batch (16 per core, 8 cores), bf16
matmuls (tokens stationary on PSUM partitions, o on the free dim), f16
stores. v2 scheduling, built from the v1 trace:
  - x is staged in HBM chunk-major ([128, 4*tl] per 1024-token chunk, c-major
    free dim) so every chunk load is 128 descriptors of 8KB instead of 512 of
    2KB; v1's 2KB descriptors capped the x queue at ~78GB/s and starved the
    PE for ~40us during L1/L2.
  - The bias column is folded into the main o-tiling (o-tiles cover idim =
    ksci+1, equal-split <=512), removing v1's separate column pass (~19us of
    PE streaming).  Per-subtile bias-column values are extracted to a
    [128, 80] SBUF buffer (scalar-engine copies), stored once, and scattered
    into y on the host (layout-only work).
  - L0/L4 outputs go to small DRAM scratches whole (host scatters them);
    their y-layout stores were ~1200 tiny (54-512B) descriptors that clog a
    DMA queue for ~25us.
  - L2 and L3 subtiles are interleaved 1:1 so the store stream is smooth
    (139GB/s) instead of a 378GB/s burst (L2) followed by 152GB/s (L3);
    stores are split across the scalar-ring queue (L1/L2) and gpsimd-ring
    queue (L3 halves), with WT3 preloaded on the vector-ring queue.
  - ~120 warmup matmuls on a zeroed tile keep the PE busy during the initial
    x DMA so the HAM clock gate ramps to full speed before real work.
"""

import numpy as np
import ml_dtypes

import concourse.bass as bass
import concourse.mybir as mybir
from concourse import bacc
from concourse.tile import TileContext
from concourse.bass_utils import run_bass_kernel_spmd

# ---------------------------------------------------------------- constants
N_CORES = 8
B = 128
BPC = B // N_CORES            # batches per core = 16
D = 512
KN = [64, 128, 256, 256, 10]
KSCI = [27, 576, 1152, 4096, 256]
IDIM = [k + 1 for k in KSCI]
START = [0, 1792, 75648, 370816, 1419648]
I_TOTAL = 1422218
TOKL = [BPC * k for k in KN]  # tokens per core per layer
NTOK = sum(TOKL)              # 11424
BBOFF = np.cumsum([0] + IDIM).tolist()  # bias table offset per layer
BBTOT = BBOFF[-1]             # 6112
TLOAD = 1024                  # tokens per x DMA chunk
BF16 = mybir.dt.bfloat16
F16 = mybir.dt.float16
F32 = mybir.dt.float32

# o-tile widths per layer: equal split of idim into ceil(idim/512) tiles
def _otw(idim):
    nt = -(-idim // 512)
    base, rem = divmod(idim, nt)
    return [base + 1] * rem + [base] * (nt - rem)
OTW = [_otw(i) for i in IDIM]

# x chunk schedule: (layer, t0, tl) in issue (= need) order
CH = [
    (0, 0, 1024), (4, 0, 160), (1, 0, 1024), (1, 1024, 1024),
    (2, 0, 1024), (2, 1024, 1024), (3, 0, 1024), (2, 2048, 1024),
    (3, 1024, 1024), (2, 3072, 1024), (3, 2048, 1024), (3, 3072, 1024),
]
CHIDX = {2: [4, 5, 7, 9], 3: [6, 8, 10, 11]}
XFOFF = np.cumsum([0] + [4 * tl for (_, _, tl) in CH]).tolist()
XFTOT = XFOFF[-1]             # 45696
# phase B subtile order: all of L2, then all of L3 (stores are split across
# two queues, so neither phase bursts past queue bandwidth; serial order
# maximizes the deadline slack for WT3 and L3's x chunks)
SCHED = [(2, i) for i in range(32)] + [(3, i) for i in range(32)]
# prefetch points: at (l, s) issue loads for these CH indices (None = WT3)
PFCH = {
    (2, 4): [5],              # L2c1
    (2, 12): [None, 7, 6],    # WT3, L2c2, L3c0
    (2, 20): [9],             # L2c3
    (3, 4): [8],              # L3c1
    (3, 12): [10],            # L3c2
    (3, 20): [11],            # L3c3
}

# bias-column scratch: L3 subtiles only (L1/L2 columns ride their scratches)
NCOLS = 32

_cache = {}
last_results = None


def _build_bass():
    nc = bacc.Bacc(
        "TRN2", target_bir_lowering=False, debug=False, num_devices=N_CORES
    )
    XF = nc.declare_dram_parameter("XF", [128, XFTOT], BF16, isOutput=False)
    WT = [
        nc.declare_dram_parameter(f"WT{l}", [128, 4 * IDIM[l]], BF16, isOutput=False)
        for l in range(5)
    ]
    BBp = nc.declare_dram_parameter("BB", [128, BBTOT], BF16, isOutput=False)
    y = nc.declare_dram_parameter("y", [BPC, I_TOTAL], F16, isOutput=True)
    Y0S = nc.declare_dram_parameter("Y0S", [128, 8 * 28], F16, isOutput=True)
    Y4S = nc.declare_dram_parameter("Y4S", [128, 2 * 257], F16, isOutput=True)
    Y1S = nc.declare_dram_parameter("Y1S", [128, 16 * 577], F16, isOutput=True)
    Y2S = nc.declare_dram_parameter("Y2S", [128, 32 * 1153], F16, isOutput=True)
    YCOL = nc.declare_dram_parameter("YCOL", [128, NCOLS], F16, isOutput=True)

    with TileContext(nc) as tc:
        with (
            tc.tile_pool(name="wt", bufs=1) as wt_pool,
            tc.tile_pool(name="bias", bufs=1) as bias_pool,
            tc.tile_pool(name="x", bufs=5) as x_pool,
            tc.tile_pool(name="outL", bufs=5) as outL_pool,
            tc.tile_pool(name="outS", bufs=2) as outS_pool,
            tc.tile_pool(name="fix", bufs=1) as fix_pool,
            tc.tile_pool(name="ps", bufs=6, space="PSUM") as ps_pool,
            tc.tile_pool(name="pw", bufs=1, space="PSUM") as pw_pool,
        ):
            # ---- PE warmup: keep the HAM activity monitor fed while the
            # first x chunk loads, so real matmuls start at full clock.
            warm = fix_pool.tile([128, 128], BF16, tag="warm")
            nc.vector.memset(warm[:, :], 0.0)
            wps = pw_pool.tile([128, 128], F32, tag="wps")
            for _ in range(64):
                nc.tensor.matmul(
                    out=wps[:, :], lhsT=warm[:, :], rhs=warm[:, :],
                    start=True, stop=True,
                )

            # ---- every load (x chunks AND tables) goes on the sync-ring
            # queue, interleaved in exact need order.  Cross-queue HBM
            # arbitration is uncontrollable; a single FIFO queue is an
            # explicit priority list.  Stores get the other two queues.
            bb = bias_pool.tile([128, BBTOT], BF16, tag="bb")
            wt = {}

            def load_tables(l, eng=None):
                eng = eng or nc.sync
                t = wt_pool.tile([128, 4 * IDIM[l]], BF16, tag=f"wt{l}")
                eng.dma_start(out=t[:, :], in_=WT[l][:, :])
                eng.dma_start(
                    out=bb[:, BBOFF[l] : BBOFF[l] + IDIM[l]],
                    in_=BBp[:, BBOFF[l] : BBOFF[l] + IDIM[l]],
                )
                wt[l] = t[:, :].rearrange("p (c o) -> p c o", c=4)

            colbuf = fix_pool.tile([128, NCOLS], F16, tag="colbuf")
            ob0 = fix_pool.tile([128, 8 * 28], F16, tag="ob0")
            ob4 = fix_pool.tile([128, 2 * 257], F16, tag="ob4")
            ob1 = fix_pool.tile([128, 16 * 577], F16, tag="ob1")
            nc.vector.memset(ob4[:, :], 0.0)  # rows >= sl never written

            ymain3 = y[:, START[3] : START[3] + KN[3] * KSCI[3]].rearrange(
                "b (k o) -> b k o", o=KSCI[3]
            )

            def load_chunk(ci, eng=None):
                eng = eng or nc.sync
                _, _, tl = CH[ci]
                xt = x_pool.tile([128, 4 * TLOAD], BF16, tag="xt")
                eng.dma_start(
                    out=xt[:, : 4 * tl], in_=XF[:, XFOFF[ci] : XFOFF[ci] + 4 * tl]
                )
                return xt[:, : 4 * tl].rearrange("p (c t) -> p c t", c=4)

            def subtile_mm(l, xv, s0, sl, drain):
                o0 = 0
                for oi, no in enumerate(OTW[l]):
                    ps = ps_pool.tile([128, 512], F32, tag="ps")
                    for dc in range(4):
                        nc.tensor.matmul(
                            out=ps[:sl, :no],
                            lhsT=xv[:, dc, s0 : s0 + sl],
                            rhs=wt[l][:, dc, o0 : o0 + no],
                            start=(dc == 0),
                            stop=(dc == 3),
                        )
                    drain(oi, o0, no, ps)
                    o0 += no

            def badd(out_ap, ps, sl, no, l, o0):
                nc.vector.tensor_add(
                    out=out_ap,
                    in0=ps[:sl, :no],
                    in1=bb[:sl, BBOFF[l] + o0 : BBOFF[l] + o0 + no],
                )

            # ---- issue ALL loads upfront on the sync ring in need order.
            # The x pool (bufs=6) lets the first 6 chunk triggers fire
            # immediately; later chunk triggers wait for buffer releases,
            # which track compute progress.  Table triggers fire eagerly and
            # their transfers serialize in this exact order on the queue.
            # Early loads are sharded across the sync AND scalar queues (the
            # scalar queue has no stores until Y1S at ~45us): two queues pull
            # measurably more aggregate HBM bandwidth than one, and phase A
            # is load-latency-bound.
            xtile = {}
            def pf(ci, eng=None):
                xtile[ci] = load_chunk(ci, eng)
            pf(0)                               # L0 x          (sync)
            load_tables(0, nc.scalar)
            load_tables(4, nc.scalar)
            pf(1, nc.scalar)                    # L4 x
            load_tables(1)                      # WT1           (sync)
            pf(2, nc.scalar)                    # L1 x 1st half
            pf(3)                               # L1 x 2nd half (sync)
            load_tables(2, nc.scalar)
            pf(4)                               # L2 chunk 0    (sync)
            # WT3 and the remaining chunks are issued from inside the phase-B
            # loop (PFCH) so the queues stay in need order.

            # ---- phase A: L0 -> scratch
            for s in range(8):
                def dr0(oi, o0, no, ps, s=s):
                    badd(ob0[:128, s * 28 + o0 : s * 28 + o0 + no], ps, 128, no, 0, o0)
                subtile_mm(0, xtile[0], s * 128, 128, dr0)
            nc.gpsimd.dma_start(out=Y0S[:, :], in_=ob0[:, :])

            # ---- L4 -> scratch (2 subtiles: 120 + 40 tokens)
            for si, (s0, sl) in enumerate(((0, 120), (120, 40))):
                def dr4(oi, o0, no, ps, si=si, sl=sl):
                    badd(
                        ob4[:sl, si * 257 + o0 : si * 257 + o0 + no],
                        ps, sl, no, 4, o0,
                    )
                subtile_mm(4, xtile[1], s0, sl, dr4)
            nc.gpsimd.dma_start(out=Y4S[:, :], in_=ob4[:, :])

            # ---- filler: L1's x arrival is HBM-bound ~1-2us after L4 ends;
            # keep the PE busy so the HAM clock gate stays at full speed.
            for _ in range(16):
                nc.tensor.matmul(
                    out=wps[:, :], lhsT=warm[:, :], rhs=warm[:, :],
                    start=True, stop=True,
                )

            # ---- L1 -> scratch (accumulated whole: 1 store of 18.5KB lines
            # instead of 2048 1.2KB store descriptors that clog the queues)
            for ch in range(2):
                xv = xtile[2 + ch]
                for si in range(8):
                    s = ch * 8 + si
                    def dr1(oi, o0, no, ps, s=s):
                        badd(
                            ob1[:128, s * 577 + o0 : s * 577 + o0 + no],
                            ps, 128, no, 1, o0,
                        )
                    subtile_mm(1, xv, si * 128, 128, dr1)
            nc.scalar.dma_start(out=Y1S[:, :], in_=ob1[:, :])

            # ---- phase B: all of L2 (scratch, 8-subtile groups), then all
            # of L3 (direct y stores in halves across both store queues).
            xcur = {}
            for l, s in SCHED:
                if s % 8 == 0:
                    xcur[l] = xtile[CHIDX[l][s // 8]]
                for ci in PFCH.get((l, s), ()):
                    if ci is None:
                        load_tables(3, nc.scalar)
                    else:
                        pf(ci)
                if l == 2:
                    j = s % 8
                    if j == 0:
                        ob2 = outS_pool.tile([128, 8 * IDIM[2]], F16, tag="ob2")
                    def dr2(oi, o0, no, ps, ob2=ob2, j=j):
                        badd(
                            ob2[:128, j * 1153 + o0 : j * 1153 + o0 + no],
                            ps, 128, no, 2, o0,
                        )
                    subtile_mm(2, xcur[2], j * 128, 128, dr2)
                    if j == 7:
                        g = s // 8
                        eng = nc.scalar if g % 2 == 0 else nc.gpsimd
                        eng.dma_start(
                            out=Y2S[:, g * 9224 : (g + 1) * 9224], in_=ob2[:, :]
                        )
                else:
                    b0, k0 = divmod(s * 128, KN[3])
                    enA = nc.scalar if s % 2 == 0 else nc.gpsimd
                    enB = nc.gpsimd if s % 2 == 0 else nc.scalar
                    ob3 = outL_pool.tile([128, IDIM[3]], F16, tag="ob3")
                    if s < 31:
                        def dr3(oi, o0, no, ps, ob3=ob3, b0=b0, k0=k0, enA=enA):
                            badd(ob3[:128, o0 : o0 + no], ps, 128, no, 3, o0)
                            if oi == 4:  # cols [0, 2277) drained
                                enA.dma_start(
                                    out=ymain3[b0, k0 : k0 + 128, 0:2048],
                                    in_=ob3[:, 0:2048],
                                )
                        subtile_mm(3, xcur[3], (s % 8) * 128, 128, dr3)
                        enB.dma_start(
                            out=ymain3[b0, k0 : k0 + 128, 2048:4096],
                            in_=ob3[:, 2048:4096],
                        )
                    else:
                        # last subtile: quarter stores to shorten the tail
                        QS = {2: (0, 1024, enA), 4: (1024, 2048, enB),
                              6: (2048, 3072, enA)}
                        def dr3(oi, o0, no, ps, ob3=ob3, b0=b0, k0=k0):
                            badd(ob3[:128, o0 : o0 + no], ps, 128, no, 3, o0)
                            if oi in QS:
                                a, b, en = QS[oi]
                                en.dma_start(
                                    out=ymain3[b0, k0 : k0 + 128, a:b],
                                    in_=ob3[:, a:b],
                                )
                        subtile_mm(3, xcur[3], (s % 8) * 128, 128, dr3)
                        enB.dma_start(
                            out=ymain3[b0, k0 : k0 + 128, 3072:4096],
                            in_=ob3[:, 3072:4096],
                        )
                    nc.scalar.copy(
                        out=colbuf[:, s : s + 1],
                        in_=ob3[:, KSCI[3] : IDIM[3]],
                    )
            nc.gpsimd.dma_start(out=YCOL[:, :], in_=colbuf[:, :])
    nc.compile()
    return nc


def _prep_inputs(inputs):
    x = np.asarray(inputs["x"], dtype=np.float32)
    xb = x.astype(ml_dtypes.bfloat16)
    shared = {}
    for l in range(5):
        W = np.asarray(inputs[f"W{l}"], dtype=np.float32).astype(ml_dtypes.bfloat16)
        # [128, 4*idim] with free dim (c, o); row p, chunk c holds W.T[c*128+p]
        shared[f"WT{l}"] = np.ascontiguousarray(
            W.T.reshape(4, 128, IDIM[l]).transpose(1, 0, 2).reshape(128, 4 * IDIM[l])
        )
    bbvec = np.concatenate(
        [np.asarray(inputs[f"b{l}"], dtype=np.float32)[: IDIM[l]] for l in range(5)]
    )
    shared["BB"] = np.ascontiguousarray(
        np.broadcast_to(bbvec.astype(ml_dtypes.bfloat16), (128, BBTOT))
    )
    off = np.cumsum([0] + KN).tolist()
    in_maps = []
    for c in range(N_CORES):
        xc = xb[c * BPC : (c + 1) * BPC]  # [16, 714, 512] bf16
        xTl = [
            np.transpose(xc[:, off[l] : off[l] + KN[l]], (2, 0, 1)).reshape(D, -1)
            for l in range(5)
        ]
        parts = []
        for l, t0, tl in CH:
            blk = xTl[l][:, t0 : t0 + tl]  # [512, tl]
            parts.append(
                blk.reshape(4, 128, tl).transpose(1, 0, 2).reshape(128, 4 * tl)
            )
        in_maps.append({"XF": np.ascontiguousarray(np.concatenate(parts, axis=1)),
                        **shared})
    return in_maps


def _assemble(res):
    y = np.empty((B, I_TOTAL), np.float32)
    for c in range(N_CORES):
        r = res.results[c]
        yc = r["y"].astype(np.float32)  # [16, I_TOTAL]
        # L0 scratch: [128, 8*28] -> tokens t=s*128+p, (b,k)=divmod(t,64)
        v0 = (
            r["Y0S"].astype(np.float32)
            .reshape(128, 8, 28).transpose(1, 0, 2).reshape(16, 64, 28)
        )
        yc[:, 0:1728] = v0[:, :, :27].reshape(16, 1728)
        yc[:, 1728:1792] = v0[:, :, 27]
        # L4 scratch: subtile 0 = tokens 0..120, subtile 1 = tokens 120..160
        v4r = r["Y4S"].astype(np.float32)  # [128, 514]
        v4 = np.concatenate(
            [v4r[:120, 0:257], v4r[:40, 257:514]], axis=0
        ).reshape(16, 10, 257)
        s4 = START[4]
        yc[:, s4 : s4 + 2560] = v4[:, :, :256].reshape(16, 2560)
        yc[:, s4 + 2560 : s4 + 2570] = v4[:, :, 256]
        # L1 scratch: [128, 16*577] -> subtile s = batch s, token p = k
        v1 = (
            r["Y1S"].astype(np.float32)
            .reshape(128, 16, 577).transpose(1, 0, 2)
        )  # [16, 128, 577]
        yc[:, 1792:75520] = v1[:, :, :576].reshape(16, 73728)
        yc[:, 75520:75648] = v1[:, :, 576]
        # L2 scratch: [128, 4*(8*1153)] -> token t = (8g+j)*128+p
        v2 = (
            r["Y2S"].astype(np.float32)
            .reshape(128, 4, 8, 1153).transpose(1, 2, 0, 3).reshape(16, 256, 1153)
        )
        yc[:, 75648:370560] = v2[:, :, :1152].reshape(16, 294912)
        yc[:, 370560:370816] = v2[:, :, 1152]
        # L3 bias columns
        ycol = r["YCOL"].astype(np.float32)  # [128, 32]
        yc[:, 1419392:1419648] = ycol.T.reshape(16, 256)
        y[c * BPC : (c + 1) * BPC] = yc
    return y


def kernel(**inputs):
    global last_results
    if "nc" not in _cache:
        _cache["nc"] = _build_bass()
    nc = _cache["nc"]
    in_maps = _prep_inputs(inputs)
    res = run_bass_kernel_spmd(nc, in_maps, list(range(N_CORES)))
    last_results = res
    return _assemble(res)
